# revision 1
# baseline (speedup 1.0000x reference)
"""Causal self-attention (RoPE) Trainium2 kernel.

Full inputs -> shard across 8 NeuronCores (tensor-parallel over heads x
data-parallel over batch) -> bass/Tile kernel per core -> host partial-sum
unshard.

Reference semantics (B=2, T=2048, C=2048, H=16, hd=128):
    qkv = x @ w_qkv ; q,k,v split ; RoPE(q,k) ; causal softmax attention ;
    y = att @ v ; out = y @ w_proj

Core c handles batch b = c//4 and 4 heads h0 = 4*(c%4).
Each core computes out_partial[T, C] = y_heads @ w_proj[rows of its heads];
host sums the 4 partials per batch.

Layout trick: RoPE pairs (2i, 2i+1) are permuted to (i, i+64) by permuting
w_qkv columns on the host, so rotate_half becomes a swap of the top/bottom
64 partitions (legal contiguous-partition DVE copies). S is invariant since
the same orthogonal permutation is applied to q and k.
"""

import sys

sys.path.insert(0, "/opt/trn_rl_repo")

import numpy as np
import ml_dtypes

import concourse.bass as bass
import concourse.mybir as mybir
import concourse.tile as tile
from concourse import bacc, bass_utils

F32 = mybir.dt.float32
BF16 = mybir.dt.bfloat16

T = 2048
C = 2048
HD = 128
NH = 16
NH_LOC = 4          # heads per core
N_CORES = 8
TQ = 512            # q-chunk (moving free dim)
KT = 128            # k-tile (S^T partition dim)
CK = 128            # contraction chunk over C
NCK = C // CK       # 16
NTQ = T // TQ       # 4
NKT = T // KT       # 16
SCALE = 1.0 / np.sqrt(HD)
MASK_NEG = -1.0e6

_compiled_nc = None


def _build():
    nc = bacc.Bacc("TRN2", target_bir_lowering=False, debug=False, num_devices=1)

    xT = nc.dram_tensor("xT", [C, T], BF16, kind="ExternalInput").ap()
    wqk = nc.dram_tensor("wqk", [C, 2 * NH_LOC * HD], BF16, kind="ExternalInput").ap()
    wv = nc.dram_tensor("wv", [C, NH_LOC * HD], BF16, kind="ExternalInput").ap()
    wp = nc.dram_tensor("wp", [NH_LOC * HD, C], BF16, kind="ExternalInput").ap()
    cosP = nc.dram_tensor("cosP", [HD, T], BF16, kind="ExternalInput").ap()
    sinP = nc.dram_tensor("sinP", [HD, T], BF16, kind="ExternalInput").ap()
    masks = nc.dram_tensor("masks", [KT, KT], F32, kind="ExternalInput").ap()
    perm = nc.dram_tensor("perm", [HD, HD], BF16, kind="ExternalInput").ap()
    out = nc.dram_tensor("out", [T, C], F32, kind="ExternalOutput").ap()

    with tile.TileContext(nc) as tc, (
        tc.tile_pool(name="persist", bufs=1)) as persist, (
        tc.tile_pool(name="weights", bufs=1)) as wpool, (
        tc.tile_pool(name="xstream", bufs=2)) as xstream, (
        tc.tile_pool(name="work", bufs=2)) as work, (
        tc.tile_pool(name="ps", bufs=1, space="PSUM")) as psp:

        # ---- persistent SBUF ----
        cos_sb = persist.tile([HD, T], BF16, tag="cos")
        sin_sb = persist.tile([HD, T], BF16, tag="sin")
        mask_sb = persist.tile([KT, KT], F32, tag="mask")
        perm_sb = persist.tile([HD, HD], BF16, tag="perm")
        ones_full = persist.tile([KT, 128], BF16, tag="ones_full")
        nc.vector.memset(ones_full[:], 1.0)

        # Q^T/K^T (d-major), V (token-major), y^T (d-major) staging
        qk_sb = [persist.tile([HD, T], BF16, name=f"qk{m}", tag=f"qk{m}") for m in range(8)]
        v_sb = [persist.tile([KT, NH_LOC * HD], BF16, name=f"v{i}", tag=f"v{i}") for i in range(NKT)]
        yt_sb = [persist.tile([HD, T], BF16, name=f"yt{h}", tag=f"yt{h}") for h in range(NH_LOC)]

        # ---- weights (whole kernel lifetime) ----
        wqk_sb = [wpool.tile([CK, 2 * NH_LOC * HD], BF16, name=f"wqk{kc}", tag=f"wqk{kc}") for kc in range(NCK)]
        wv_sb = [wpool.tile([CK, NH_LOC * HD], BF16, name=f"wv{kc}", tag=f"wv{kc}") for kc in range(NCK)]
        wp_sb = [wpool.tile([HD, C], BF16, name=f"wp{h}", tag=f"wp{h}") for h in range(NH_LOC)]
        # interleave wqk with the first xT chunk so the first QK
        # accumulation group can start as soon as pair 0 lands
        xc0 = [xstream.tile([CK, TQ], BF16, name=f"xc{kc}", tag=f"xc{kc}") for kc in range(NCK)]
        for kc in range(NCK):
            nc.gpsimd.dma_start(wqk_sb[kc][:], wqk[kc * CK:(kc + 1) * CK, :])
            nc.sync.dma_start(xc0[kc][:], xT[kc * CK:(kc + 1) * CK, 0:TQ])
            nc.gpsimd.dma_start(wv_sb[kc][:], wv[kc * CK:(kc + 1) * CK, :])
        nc.gpsimd.dma_start(cos_sb[:], cosP)
        nc.gpsimd.dma_start(sin_sb[:], sinP)
        nc.gpsimd.dma_start(perm_sb[:], perm)
        nc.gpsimd.dma_start(mask_sb[:], masks)
        for h in range(NH_LOC):
            nc.gpsimd.dma_start(wp_sb[h][:], wp[h * HD:(h + 1) * HD, :])

        # PSUM tags (shared across phases via slot cycling, 8 banks total):
        #   "a": QK-proj psums / S^T tiles / out-proj psums   (3 bufs)
        #   "b": V-proj psums / y^T accumulators              (2 bufs)
        #   "c": rope shuffles / warmup / rowsum-broadcast    (2 bufs)
        def ps_a():
            return psp.tile([128, TQ], F32, name="psa", tag="a", bufs=5)
        def ps_b():
            return psp.tile([128, TQ], F32, name="psb", tag="b", bufs=2)
        def ps_c():
            return psp.tile([128, TQ], F32, name="psc", tag="c", bufs=1)

        for jq in range(NTQ):
            tsl = slice(jq * TQ, (jq + 1) * TQ)

            # ======== QKV projection for token chunk jq ========
            if jq == 0:
                xT_sb = xc0
            else:
                xT_sb = [xstream.tile([CK, TQ], BF16, name=f"xc{kc}", tag=f"xc{kc}") for kc in range(NCK)]
                for kc in range(NCK):
                    nc.sync.dma_start(xT_sb[kc][:], xT[kc * CK:(kc + 1) * CK, tsl])

            for m in range(8):
                ps = ps_a()
                for kc in range(NCK):
                    nc.tensor.matmul(
                        ps[:],
                        wqk_sb[kc][:, m * 128:(m + 1) * 128],
                        xT_sb[kc][:],
                        start=(kc == 0),
                        stop=(kc == NCK - 1),
                    )
                dst = qk_sb[m][:, tsl]
                nc.scalar.activation(dst, ps[:], mybir.ActivationFunctionType.Copy)
                # RoPE in place: X = X*cos + PERM @ (X*sinSwap)
                tmp = work.tile([HD, TQ], BF16, tag="rope", bufs=3)
                nc.vector.tensor_mul(tmp[:], dst, sin_sb[:, tsl])
                pp = ps_c()
                nc.tensor.matmul(pp[:], perm_sb[:], tmp[:], start=True, stop=True)
                nc.vector.tensor_mul(dst, dst, cos_sb[:, tsl])
                nc.vector.tensor_add(dst, dst, pp[:])

            for s4 in range(4):
                it = jq * 4 + s4
                ps = ps_b()
                for kc in range(NCK):
                    nc.tensor.matmul(
                        ps[:],
                        xT_sb[kc][:, s4 * KT:(s4 + 1) * KT],
                        wv_sb[kc][:],
                        start=(kc == 0),
                        stop=(kc == NCK - 1),
                    )
                nc.vector.tensor_copy(v_sb[it][:], ps[:])

            # ======== attention for q-chunk jq, all local heads ========
            nk = 4 * jq + 4
            for h in range(NH_LOC):
                yt_ps = ps_b()
                rs_ps = ps_c()
                for ik in range(nk):
                    r = ik - 4 * jq
                    # columns q < 128*r of this S^T tile are fully masked
                    qo = 128 * r if r >= 1 else 0
                    s_ps = ps_a()
                    nc.tensor.matmul(
                        s_ps[:, qo:],
                        qk_sb[4 + h][:, ik * KT:(ik + 1) * KT],
                        qk_sb[h][:, jq * TQ + qo:(jq + 1) * TQ],
                        start=True,
                        stop=True,
                    )
                    if r >= 0:
                        # only the first 128 kept columns straddle the
                        # diagonal; columns beyond qo+128 are fully valid
                        nc.vector.tensor_add(
                            s_ps[:, qo:qo + KT], s_ps[:, qo:qo + KT], mask_sb[:]
                        )
                    p_t = work.tile([KT, TQ], BF16, tag="p", bufs=8)
                    nc.scalar.activation(
                        p_t[:, qo:], s_ps[:, qo:],
                        mybir.ActivationFunctionType.Exp,
                        scale=float(SCALE),
                    )
                    nc.tensor.matmul(
                        yt_ps[:, qo:],
                        v_sb[ik][:, h * HD:(h + 1) * HD],
                        p_t[:, qo:],
                        start=(ik == 0),
                        stop=(ik == nk - 1),
                    )
                    nc.tensor.matmul(
                        rs_ps[:, qo:],
                        ones_full[:],
                        p_t[:, qo:],
                        start=(ik == 0),
                        stop=(ik == nk - 1),
                    )
                recip_sb = work.tile([128, TQ], F32, tag="recip")
                nc.vector.reciprocal_approx_fast(recip_sb[:], rs_ps[:])
                nc.vector.tensor_mul(yt_sb[h][:, tsl], yt_ps[:], recip_sb[:])

            # ======== output projection for token chunk jq ========
            for s4 in range(4):
                it = jq * 4 + s4
                for cc in range(4):
                    ps = ps_a()
                    for h in range(NH_LOC):
                        nc.tensor.matmul(
                            ps[:],
                            yt_sb[h][:, it * KT:(it + 1) * KT],
                            wp_sb[h][:, cc * TQ:(cc + 1) * TQ],
                            start=(h == 0),
                            stop=(h == NH_LOC - 1),
                        )
                    ot = work.tile([128, TQ], F32, tag="ot")
                    nc.scalar.activation(
                        ot[:], ps[:], mybir.ActivationFunctionType.Copy
                    )
                    nc.scalar.dma_start(
                        out[it * KT:(it + 1) * KT, cc * TQ:(cc + 1) * TQ],
                        ot[:],
                    )

    nc.compile()
    return nc


def _get_nc():
    global _compiled_nc
    if _compiled_nc is None:
        _compiled_nc = _build()
    return _compiled_nc


def _rope_tables():
    t = np.arange(T, dtype=np.float64)
    inv_freq = 1.0 / (10000.0 ** (np.arange(0, HD, 2, dtype=np.float64) / HD))
    freqs = np.outer(t, inv_freq)            # [T, 64]
    cos_half = np.cos(freqs).T               # [64, T]
    sin_half = np.sin(freqs).T
    cosP = np.concatenate([cos_half, cos_half], axis=0)      # [128, T]
    # tmp = X*sinSwap is computed in the UNshuffled frame, then partitions are
    # swapped by PERM: row j<64 holds +sin_half[j] (lands at j+64 after swap),
    # row j+64 holds -sin_half[j] (lands at j).
    sinSwap = np.concatenate([sin_half, -sin_half], axis=0)
    return (cosP.astype(ml_dtypes.bfloat16), sinSwap.astype(ml_dtypes.bfloat16))


def _mask_tiles():
    kl = np.arange(KT)[:, None]              # [128, 1]
    c = np.arange(KT)[None, :]               # [1, 128]
    return np.where(kl <= c, 0.0, MASK_NEG).astype(np.float32)


def _perm_matrix():
    p = np.zeros((HD, HD), dtype=ml_dtypes.bfloat16)
    i = np.arange(64)
    p[i + 64, i] = 1.0
    p[i, i + 64] = 1.0
    return p


def _head_perm(h0):
    """Permuted q/k columns for heads h0..h0+3: pairs (2i,2i+1)->(i,i+64)."""
    cols = []
    for h in range(h0, h0 + NH_LOC):
        base = h * HD
        cols.extend(base + 2 * np.arange(64))
        cols.extend(base + 2 * np.arange(64) + 1)
    return np.array(cols)


def _make_in_maps(x, w_qkv, w_proj):
    x = np.asarray(x)
    w_qkv = np.asarray(w_qkv)
    w_proj = np.asarray(w_proj)
    B = x.shape[0]
    assert x.shape == (B, T, C) and B == 2

    cosP, sinP = _rope_tables()
    masks = _mask_tiles()
    permM = _perm_matrix()

    bf = ml_dtypes.bfloat16
    xT_b = [np.ascontiguousarray(x[b].T).astype(bf) for b in range(B)]

    in_maps = []
    for c in range(N_CORES):
        b = c // 4
        h0 = NH_LOC * (c % 4)
        perm = _head_perm(h0)
        wqk_c = np.concatenate(
            [w_qkv[:, perm], w_qkv[:, C + perm]], axis=1
        ).astype(bf)                                        # [C, 1024]
        vcols = np.arange(h0 * HD, (h0 + NH_LOC) * HD)
        wv_c = w_qkv[:, 2 * C + vcols].astype(bf)           # [C, 512]
        wp_c = w_proj[h0 * HD:(h0 + NH_LOC) * HD, :].astype(bf)  # [512, C]
        in_maps.append({
            "xT": xT_b[b],
            "wqk": np.ascontiguousarray(wqk_c),
            "wv": np.ascontiguousarray(wv_c),
            "wp": np.ascontiguousarray(wp_c),
            "cosP": cosP,
            "sinP": sinP,
            "masks": masks,
            "perm": permM,
        })
    return in_maps


def _reduce_out(results):
    out = np.zeros((2, T, C), dtype=np.float32)
    for c in range(N_CORES):
        out[c // 4] += results[c]["out"]
    return out


_cached_exec = None


def _get_cached_exec():
    """Build (once) a jitted SPMD executable for the compiled Bass module.

    Mirrors bass2jax.run_bass_via_pjrt's multi-core path but caches the
    jitted shard_map callable so repeat kernel() calls skip retracing.
    """
    global _cached_exec
    if _cached_exec is not None:
        return _cached_exec
    import jax
    from jax.experimental.shard_map import shard_map
    from jax.sharding import Mesh, PartitionSpec
    from concourse import bass2jax

    nc = _get_nc()
    bass2jax.install_neuronx_cc_hook()
    in_names, out_names, out_avals = [], [], []
    for alloc in nc.m.functions[0].allocations:
        if not isinstance(alloc, mybir.MemoryLocationSet):
            continue
        name = alloc.memorylocations[0].name
        if alloc.kind == "ExternalInput":
            in_names.append(name)
        elif alloc.kind == "ExternalOutput":
            out_names.append(name)
            out_avals.append(
                jax.core.ShapedArray(
                    tuple(alloc.tensor_shape), mybir.dt.np(alloc.dtype)
                )
            )
    n_params = len(in_names)
    all_names = tuple(in_names) + tuple(out_names)
    donate = tuple(range(n_params, n_params + len(out_names)))

    def _body(*args):
        outs = bass2jax._bass_exec_p.bind(
            *args,
            out_avals=tuple(out_avals),
            in_names=all_names,
            out_names=tuple(out_names),
            lowering_input_output_aliases=(),
            sim_require_finite=True,
            sim_require_nnan=True,
            nc=nc,
        )
        return tuple(outs)

    devices = jax.devices()[:N_CORES]
    mesh = Mesh(np.asarray(devices), ("core",))
    nin = n_params + len(out_names)
    sharded = jax.jit(
        shard_map(
            _body,
            mesh=mesh,
            in_specs=(PartitionSpec("core"),) * nin,
            out_specs=(PartitionSpec("core"),) * len(out_names),
            check_rep=False,
        ),
        donate_argnums=donate,
        keep_unused=True,
    )
    _cached_exec = (sharded, in_names, out_names, out_avals)
    return _cached_exec


def _run_cached(in_maps):
    sharded, in_names, out_names, out_avals = _get_cached_exec()
    concat_in = [
        np.concatenate([np.asarray(in_maps[c][k]) for c in range(N_CORES)], axis=0)
        for k in in_names
    ]
    concat_zeros = [
        np.zeros((N_CORES * av.shape[0], *av.shape[1:]), av.dtype)
        for av in out_avals
    ]
    out_arrs = sharded(*concat_in, *concat_zeros)
    return [
        {
            k: np.asarray(out_arrs[i]).reshape(N_CORES, *out_avals[i].shape)[c]
            for i, k in enumerate(out_names)
        }
        for c in range(N_CORES)
    ]


def kernel(x, w_qkv, w_proj):
    nc = _get_nc()
    in_maps = _make_in_maps(x, w_qkv, w_proj)
    try:
        results = _run_cached(in_maps)
    except Exception:
        res = bass_utils.run_bass_kernel_spmd(nc, in_maps, list(range(N_CORES)))
        results = res.results
    return _reduce_out(results)

